# revision 9
# baseline (speedup 1.0000x reference)
"""DeTPP loss kernel for 8 TRN2 NeuronCores (batch-parallel SPMD Bass/Tile).

Strategy: shard along batch B (8 per core). Per core, on device:
  - batched indirect-DMA gathers (SWDGE desc-gen ~0.34ns/desc + ~1us/instr):
      rec:   [R,24] f32 rows (time/amount/cat windows, out_time/out_amount)
      rows:  [R,1024] bf16 logits rows (k-major), 5 groups (2,2,4,4,4 tiles)
      picks: 4-contiguous-element runs from a c-major bf16 logits copy,
             8192 offsets computed on DVE from the gathered cat windows
  - ACT: exp per group; DVE: bf16 fold-tree + reduce for per-k sums over C,
    L1/cost assembly split per half (overlaps second-half exp); CE's lse
    joins perm-invariantly at the end (single Ln table switch)
  - 24-permutation totals via PE transpose + block-diag matmul,
    segmented min-reduce, masked sum; host sums per-core (sum, count).
"""
import sys

sys.path.insert(0, '/opt/trn_rl_repo')

import itertools
import numpy as np
import ml_dtypes

BF16 = ml_dtypes.bfloat16

L, B, I, K, C = 1024, 64, 256, 4, 256
BS = B // 8            # batch per core
R = L * BS             # rows per core (8192), row id r = l*BS + b
N = I * BS             # gathered items per core (2048)
NT = N // 128          # 16 tiles; tile t holds item n = p*NT + t (p = partition)
KC = K * C             # 1024
RECW = 24              # rec row width (f32): t5(5) a4(4) c4(4) ot(4) oa(4) pad(3)
F_T5, F_A4, F_C4, F_OT, F_OA = 0, 5, 9, 13, 17
PERMS = np.array(list(itertools.permutations(range(K))), dtype=np.int32)
NP_ = PERMS.shape[0]   # 24
GRP = [2, 2, 4, 4, 4]  # tiles per row-gather group (h0 = g0+g1+g2)


def _host_prep(core, time, amount, out_time, out_amount, out_cat_logits, cat,
               lengths, indices, consts):
    bsl = slice(core * BS, (core + 1) * BS)
    lg = out_cat_logits[:, bsl].astype(BF16)                   # (L, BS, K, C)
    aug1 = np.ascontiguousarray(lg).reshape(R, KC)
    aug2 = np.ascontiguousarray(lg.transpose(0, 1, 3, 2)).reshape(R * KC, 1)
    t5 = np.stack([np.roll(time[:, bsl], -j, axis=0) for j in range(5)], axis=2)
    a4 = np.stack([np.roll(amount[:, bsl], -j, axis=0) for j in range(1, 5)], axis=2)
    c4 = np.stack([np.roll(cat[:, bsl], -j, axis=0) for j in range(1, 5)],
                  axis=2).astype(np.float32)
    rect = np.concatenate(
        [t5, a4, c4, out_time[:, bsl], out_amount[:, bsl],
         np.zeros((L, BS, 3), np.float32)], axis=2).reshape(R, RECW)
    return {
        "aug1": aug1,
        "aug2": aug2,
        "rect": np.ascontiguousarray(rect, dtype=np.float32),
        "idx_f": np.ascontiguousarray(indices[:, bsl]).reshape(-1),
        "len_rep": np.tile(lengths[bsl][np.arange(NT) % BS].astype(np.float32),
                           (128, 1)),
        **consts,
    }


def _make_consts():
    pmat1 = np.zeros((K * K, NP_), np.float32)
    for p in range(NP_):
        for k in range(K):
            pmat1[k * K + PERMS[p, k], p] = 1.0
    pmat = np.zeros((128, 8 * NP_), np.float32)
    for tblk in range(8):
        pmat[tblk * 16:(tblk + 1) * 16, tblk * NP_:(tblk + 1) * NP_] = pmat1
    return {
        "bpat": np.tile((np.arange(NT) % BS).astype(np.int32), (128, 1)),
        "pmat": pmat,
        "ident": np.eye(128, dtype=np.float32),
        "ones1": np.ones((128, 1), np.float32),
    }


def _build(nc, bass, mybir, tile):
    AP = bass.AP
    dt = mybir.dt
    Alu = mybir.AluOpType
    Act = mybir.ActivationFunctionType

    aug1 = nc.dram_tensor("aug1", [R, KC], dt.bfloat16, kind="ExternalInput")
    aug2 = nc.dram_tensor("aug2", [R * KC, 1], dt.bfloat16, kind="ExternalInput")
    rect = nc.dram_tensor("rect", [R, RECW], dt.float32, kind="ExternalInput")
    idx_f = nc.dram_tensor("idx_f", [1, N], dt.int32, kind="ExternalInput")
    len_rep = nc.dram_tensor("len_rep", [128, NT], dt.float32, kind="ExternalInput")
    bpat = nc.dram_tensor("bpat", [128, NT], dt.int32, kind="ExternalInput")
    pmat = nc.dram_tensor("pmat", [128, 8 * NP_], dt.float32, kind="ExternalInput")
    ident = nc.dram_tensor("ident", [128, 128], dt.float32, kind="ExternalInput")
    ones1 = nc.dram_tensor("ones1", [128, 1], dt.float32, kind="ExternalInput")
    out = nc.dram_tensor("out", [2, 1], dt.float32, kind="ExternalOutput")

    def dview(t, off, pattern):
        return AP(t.ap().tensor, off, pattern)

    def vw(ap2, off, dims):
        # keep partition dim, replace free dims (strides/sizes in elements)
        a = ap2
        return AP(a.tensor, a.offset + off, [list(a.ap[0])] + [list(d) for d in dims])

    with tile.TileContext(nc) as tc:
        with (
            tc.tile_pool(name="consts", bufs=1) as cpool,
            tc.tile_pool(name="gbuf", bufs=len(GRP)) as gpool,
            tc.tile_pool(name="ebuf", bufs=2) as epool,
            tc.tile_pool(name="fbuf", bufs=2) as fpool,
            tc.tile_pool(name="work", bufs=3) as wpool,
            tc.tile_pool(name="small", bufs=1) as spool,
            tc.tile_pool(name="psum", bufs=1, space="PSUM") as ppool,
        ):
            # ---- index load first (gates everything); consts on other queues
            idxt = spool.tile([128, NT], dt.int32)
            nc.sync.dma_start(idxt[:], dview(idx_f, 0, [[NT, 128], [1, NT]]))
            c_len = cpool.tile([128, NT], dt.float32)
            nc.scalar.dma_start(c_len[:], len_rep.ap())
            c_bpat = cpool.tile([128, NT], dt.int32)
            nc.scalar.dma_start(c_bpat[:], bpat.ap())
            c_pmat8 = cpool.tile([128, 8 * NP_], dt.float32)
            nc.sync.dma_start(c_pmat8[:], pmat.ap())
            c_id = cpool.tile([128, 128], dt.float32)
            nc.sync.dma_start(c_id[:], ident.ap())
            c_ones = cpool.tile([128, 1], dt.float32)
            nc.sync.dma_start(c_ones[:], ones1.ap())

            rows8 = spool.tile([128, NT], dt.int32)
            nc.vector.tensor_scalar(out=rows8[:], in0=idxt[:], scalar1=BS,
                                    scalar2=None, op0=Alu.mult)
            rowi = spool.tile([128, NT], dt.int32)
            nc.vector.tensor_tensor(out=rowi[:], in0=rows8[:], in1=c_bpat[:],
                                    op=Alu.add)
            idxf = spool.tile([128, NT], dt.float32)
            nc.vector.tensor_copy(out=idxf[:], in_=idxt[:])
            valid = spool.tile([128, NT], dt.float32)
            nc.vector.scalar_tensor_tensor(out=valid[:], in0=idxf[:], scalar=float(K),
                                           in1=c_len[:], op0=Alu.add, op1=Alu.is_lt)
            cnt = spool.tile([128, 1], dt.float32)
            nc.vector.tensor_reduce(out=cnt[:], in_=valid[:],
                                    axis=mybir.AxisListType.X, op=Alu.add)

            # ---- gathers: rows g0, g1, rec, g2, g3, picks, g4
            ra = spool.tile([128, NT * RECW], dt.float32)
            Gs = []
            gbase = [sum(GRP[:i]) for i in range(len(GRP))]

            def issue_rows(g):
                w = GRP[g] * KC
                G = gpool.tile([128, w], dt.bfloat16, tag=f"G{g}")
                nc.gpsimd.indirect_dma_start(
                    out=G[:], out_offset=None, in_=aug1.ap(),
                    in_offset=bass.IndirectOffsetOnAxis(
                        ap=rowi[:, gbase[g]:gbase[g] + GRP[g]], axis=0))
                Gs.append(G)

            issue_rows(0)
            issue_rows(1)
            nc.gpsimd.indirect_dma_start(
                out=ra[:], out_offset=None, in_=rect.ap(),
                in_offset=bass.IndirectOffsetOnAxis(ap=rowi[:], axis=0))
            issue_rows(2)
            issue_rows(3)

            # pick offsets from gathered cat windows (f32 -> i32)
            cati = spool.tile([128, NT * K], dt.int32)
            nc.vector.tensor_copy(
                out=cati[:].rearrange("p (t j) -> p t j", j=K),
                in_=vw(ra[:], F_C4, [[RECW, NT], [1, K]]))
            po = spool.tile([128, NT * K], dt.int32)
            nc.vector.tensor_scalar(out=po[:], in0=cati[:], scalar1=K,
                                    scalar2=None, op0=Alu.mult)
            rowikc = spool.tile([128, NT], dt.int32)
            nc.vector.tensor_scalar(out=rowikc[:], in0=rowi[:], scalar1=KC,
                                    scalar2=None, op0=Alu.mult)
            nc.vector.tensor_tensor(
                out=po[:].rearrange("p (t j) -> p t j", j=K),
                in0=po[:].rearrange("p (t j) -> p t j", j=K),
                in1=vw(rowikc[:], 0, [[1, NT], [0, K]]), op=Alu.add)

            pickt = spool.tile([128, NT * K * K], dt.bfloat16)
            nc.gpsimd.indirect_dma_start(
                out=pickt[:], out_offset=None, in_=aug2.ap(),
                in_offset=bass.IndirectOffsetOnAxis(ap=po[:], axis=0))

            issue_rows(4)

            # ---- exp per group + per-(t,k) sums over C via bf16 fold tree
            s4all = spool.tile([128, NT * K], dt.float32)

            def exp_sums(g):
                nt = GRP[g]
                w = nt * KC
                E = epool.tile([128, w], dt.bfloat16, tag="E")
                nc.scalar.activation(out=E[:], in_=Gs[g][:], func=Act.Exp)
                F = fpool.tile([128, w // 2], dt.bfloat16, tag="F")
                nk = nt * K
                # fold c: 256 -> 128 -> 64 -> 32, then reduce
                nc.vector.tensor_tensor(
                    out=F[:].rearrange("p (s c) -> p s c", c=128),
                    in0=vw(E[:], 0, [[C, nk], [1, 128]]),
                    in1=vw(E[:], 128, [[C, nk], [1, 128]]), op=Alu.add)
                nc.vector.tensor_tensor(
                    out=vw(F[:], 0, [[128, nk], [1, 64]]),
                    in0=vw(F[:], 0, [[128, nk], [1, 64]]),
                    in1=vw(F[:], 64, [[128, nk], [1, 64]]), op=Alu.add)
                nc.vector.tensor_tensor(
                    out=vw(F[:], 0, [[128, nk], [1, 32]]),
                    in0=vw(F[:], 0, [[128, nk], [1, 32]]),
                    in1=vw(F[:], 32, [[128, nk], [1, 32]]), op=Alu.add)
                nc.vector.tensor_reduce(
                    out=s4all[:, gbase[g] * K:(gbase[g] + nt) * K],
                    in_=vw(F[:], 0, [[128, nk], [1, 32]]),
                    axis=mybir.AxisListType.X, op=Alu.add)

            # ---- per-half cost assembly + PE (cost = l1t + l1a - pick)
            acc = spool.tile([128, NT], dt.float32)
            costall = spool.tile([128, NT * K * K], dt.float32)
            d2 = spool.tile([128, NT * K * K], dt.float32)
            dtt = spool.tile([128, NT * K], dt.float32)

            def half_cost(h):
                t0 = h * (NT // 2)
                sl = slice(t0 * K * K, (t0 + NT // 2) * K * K)
                ts = NT // 2
                rao = t0 * RECW
                nc.vector.tensor_tensor(
                    out=vw(dtt[:], t0 * K, [[K, ts], [1, K]]),
                    in0=vw(ra[:], rao + F_T5 + 1, [[RECW, ts], [1, K]]),
                    in1=vw(ra[:], rao + F_T5, [[RECW, ts], [0, K]]),
                    op=Alu.subtract)
                cv = vw(costall[:], t0 * K * K, [[K * K, ts], [K, K], [1, K]])
                nc.vector.tensor_tensor(
                    out=cv,
                    in0=vw(ra[:], rao + F_OT, [[RECW, ts], [1, K], [0, K]]),
                    in1=vw(dtt[:], t0 * K, [[K, ts], [0, K], [1, K]]),
                    op=Alu.subtract)
                nc.vector.scalar_tensor_tensor(
                    out=costall[:, sl], in0=costall[:, sl], scalar=-1.0,
                    in1=costall[:, sl], op0=Alu.mult, op1=Alu.max)
                nc.vector.tensor_tensor(
                    out=vw(d2[:], t0 * K * K, [[K * K, ts], [K, K], [1, K]]),
                    in0=vw(ra[:], rao + F_OA, [[RECW, ts], [1, K], [0, K]]),
                    in1=vw(ra[:], rao + F_A4, [[RECW, ts], [0, K], [1, K]]),
                    op=Alu.subtract)
                nc.vector.scalar_tensor_tensor(
                    out=d2[:, sl], in0=d2[:, sl], scalar=-1.0,
                    in1=d2[:, sl], op0=Alu.mult, op1=Alu.max)
                nc.vector.tensor_tensor(out=costall[:, sl], in0=costall[:, sl],
                                        in1=d2[:, sl], op=Alu.add)
                nc.vector.tensor_tensor(
                    out=cv, in0=cv,
                    in1=vw(pickt[:], t0 * K * K, [[K * K, ts], [1, K], [K, K]]),
                    op=Alu.subtract)
                pT = ppool.tile([128, 128], dt.float32, tag=f"pT{h}")
                nc.tensor.transpose(out=pT[:], in_=costall[:, sl],
                                    identity=c_id[:])
                cT = spool.tile([128, 128], dt.float32, tag=f"cT{h}")
                nc.vector.tensor_copy(out=cT[:], in_=pT[:])
                ptot = ppool.tile([128, 8 * NP_], dt.float32, tag=f"ptot{h}")
                nc.tensor.matmul(out=ptot[:], lhsT=cT[:], rhs=c_pmat8[:],
                                 start=True, stop=True)
                mint8 = wpool.tile([128, 8], dt.float32, tag=f"mint{h}")
                nc.vector.tensor_reduce(
                    out=mint8[:], in_=ptot[:].rearrange("p (t q) -> p t q", q=NP_),
                    axis=mybir.AxisListType.X, op=Alu.min)
                nc.vector.tensor_tensor(out=acc[:, h * 8:(h + 1) * 8], in0=mint8[:],
                                        in1=valid[:, h * 8:(h + 1) * 8], op=Alu.mult)

            exp_sums(0)
            exp_sums(1)
            exp_sums(2)
            exp_sums(3)
            half_cost(0)
            exp_sums(4)
            half_cost(1)

            # ---- lse: single Ln at the end, perm-invariant join
            lnall = spool.tile([128, NT * K], dt.float32)
            nc.scalar.activation(out=lnall[:], in_=s4all[:], func=Act.Ln)
            sall = spool.tile([128, NT], dt.float32)
            nc.vector.tensor_reduce(
                out=sall[:], in_=lnall[:].rearrange("p (t k) -> p t k", k=K),
                axis=mybir.AxisListType.X, op=Alu.add)
            nc.vector.tensor_tensor(out=sall[:], in0=sall[:], in1=valid[:],
                                    op=Alu.mult)

            # ---- final reduction
            nc.vector.tensor_tensor(out=acc[:], in0=acc[:], in1=sall[:], op=Alu.add)
            pair = spool.tile([128, 2], dt.float32)
            nc.vector.tensor_reduce(out=pair[:, 0:1], in_=acc[:],
                                    axis=mybir.AxisListType.X, op=Alu.add)
            nc.vector.tensor_copy(out=pair[:, 1:2], in_=cnt[:])
            pf = ppool.tile([2, 1], dt.float32, tag="pf")
            nc.tensor.matmul(out=pf[:], lhsT=pair[:], rhs=c_ones[:],
                             start=True, stop=True)
            sb = spool.tile([2, 1], dt.float32)
            nc.vector.tensor_copy(out=sb[:], in_=pf[:])
            nc.sync.dma_start(out.ap(), sb[:])
    return nc


NCORES = 8
_COMPILED = {}


def _get_compiled():
    if "nc" not in _COMPILED:
        import concourse.bacc as bacc
        import concourse.bass as bass
        import concourse.mybir as mybir
        import concourse.tile as tile
        nc = bacc.Bacc("TRN2", target_bir_lowering=False, debug=False,
                       num_devices=NCORES)
        _build(nc, bass, mybir, tile)
        nc.compile()
        _COMPILED["nc"] = nc
    return _COMPILED["nc"]


def kernel(time, amount, out_time, out_amount, out_cat_logits, cat, lengths,
           indices):
    from concourse.bass_utils import run_bass_kernel_spmd

    time = np.asarray(time, dtype=np.float32)
    amount = np.asarray(amount, dtype=np.float32)
    out_time = np.asarray(out_time, dtype=np.float32)
    out_amount = np.asarray(out_amount, dtype=np.float32)
    out_cat_logits = np.asarray(out_cat_logits, dtype=np.float32)
    cat = np.asarray(cat, dtype=np.int32)
    lengths = np.asarray(lengths, dtype=np.int32)
    indices = np.asarray(indices, dtype=np.int32)

    nc = _get_compiled()
    consts = _make_consts()
    in_maps = [
        _host_prep(c, time, amount, out_time, out_amount, out_cat_logits, cat,
                   lengths, indices, consts)
        for c in range(NCORES)
    ]
    res = run_bass_kernel_spmd(nc, in_maps, core_ids=list(range(NCORES)))
    ls = sum(float(res.results[c]["out"][0, 0]) for c in range(NCORES))
    cn = sum(float(res.results[c]["out"][1, 0]) for c in range(NCORES))
    return np.float32(ls / (cn * K))
